# revision 8
# baseline (speedup 1.0000x reference)
"""Trainium2 Bass kernel for nn_Concurrent_13623636263650 (gnn_message_passing).

Math (per batch sample, N=2000 nodes, C=64):
  u      = res / ||res||_row                  (N, C)  unit rows
  raw    = u @ u.T with zeroed diag = u@u.T - I   (symmetric)
  gmax   = max(raw flat incl diag zeros), gmin = min(...)
  rng    = gmax - gmin
  rowsum = (u@t - 1 - N*gmin)/rng,  t = sum_n u_n
  d      = rowsum^-1/2
  h      = d * origin;  q = u.T@h;  sv = sum_n h_n
  x_g1   = d * ((u@q - h) - gmin*sv) / rng
  out    = tanh(M @ Wflat + origin @ bpool).T,  M[n, d*16+i] = origin[n,d]*x_g1[n,i]

Only gmin/gmax need the full N^2 gram.  The gram runs in fp16 on the PE
(1 cyc/row + fast weight load), tiles are cast fp32->fp16 on ACT and
min/max-scanned on DVE at the 2x_1p packed rate.  The diagonal is removed
by accumulating -I into the diagonal gram tile on the PE (no mask multiply).
Everything else uses small factorized matmuls in fp16.

Node mapping n = p*16 + i (p = sbuf partition, i = chunk) makes the input
DMA fully contiguous (4KB per (sample, partition)); the output permutation
is folded into the final matmul's moving-operand access pattern (5 output
tiles of 400 columns with free dims [[1,25],[125,16]]).

Sharding: batch 16 across 8 cores (2 samples per core), SPMD program.
"""

import numpy as np
from contextlib import ExitStack

import concourse.bass as bass
import concourse.bacc as bacc
import concourse.tile as tile
from concourse import mybir
from concourse import bass_isa
from concourse.masks import make_identity

B, NN, C = 16, 2000, 64
F, O = 16, 32
NCORES = 8
SPC = B // NCORES          # samples per core

FP32 = mybir.dt.float32
FP16 = mybir.dt.float16
AX = mybir.AxisListType
AL = mybir.AluOpType
AF = mybir.ActivationFunctionType

P = 125                    # partitions per node chunk
NCH = NN // P              # node chunks (16)
BLK = 500                  # gram tile width
NT = NN // BLK             # gram tiles per row (4)
OTW = 400                  # output tile width (25 p-values x 16 i-values)
NOT = NN // OTW            # output tiles (5)


def ap_view(sl, dims):
    """AP over slice `sl` keeping its partition dim/offset, with explicit
    [stride, count] free dims (element units; stride 0 broadcasts)."""
    return bass.AP(tensor=sl.tensor, offset=sl.offset, ap=[sl.ap[0]] + list(dims))


def build_program(nc):
    res_d = nc.dram_tensor("res", [SPC, NN, C], FP32, kind="ExternalInput").ap()
    org_d = nc.dram_tensor("origin", [SPC, NN, F], FP32, kind="ExternalInput").ap()
    wp_d = nc.dram_tensor("wpool", [F, F, O], FP32, kind="ExternalInput").ap()
    bp_d = nc.dram_tensor("bpool", [F, O], FP32, kind="ExternalInput").ap()
    out_d = nc.dram_tensor("out", [SPC, O, NN], FP32, kind="ExternalOutput").ap()

    with tile.TileContext(nc) as tc, ExitStack() as ctx:
        consts = ctx.enter_context(tc.tile_pool(name="consts", bufs=1))
        big = ctx.enter_context(tc.tile_pool(name="big", bufs=1))
        scal = ctx.enter_context(tc.tile_pool(name="scal", bufs=1))
        sg = ctx.enter_context(tc.tile_pool(name="sg", bufs=2))
        # psum pools (8 banks total): gp 2x2 + tp 2x1 + tail 2x1
        pg = ctx.enter_context(tc.tile_pool(name="pg", bufs=2, space="PSUM"))
        ptp = ctx.enter_context(tc.tile_pool(name="ptp", bufs=2, space="PSUM"))
        ptl = ctx.enter_context(tc.tile_pool(name="ptl", bufs=2, space="PSUM"))

        # ---------------- loads & constants ----------------
        res32 = big.tile([P, SPC, NCH, C], FP32)
        org32 = big.tile([P, SPC, NCH, F], FP32)
        for s in range(SPC):
            nc.sync.dma_start(
                out=res32[:, s], in_=res_d[s].rearrange("(p i) c -> p i c", p=P))
            nc.sync.dma_start(
                out=org32[:, s], in_=org_d[s].rearrange("(p i) c -> p i c", p=P))
        w32 = consts.tile([128, 2, O], FP32)
        nc.sync.dma_start(out=w32, in_=wp_d.rearrange("(k d) i o -> (d i) k o", k=2))
        bp32 = consts.tile([F, O], FP32)
        nc.sync.dma_start(out=bp32, in_=bp_d)

        ident16 = consts.tile([P, P], FP16)
        make_identity(nc, ident16)
        negid16 = consts.tile([P, P], FP16)
        nc.vector.tensor_scalar(out=negid16, in0=ident16, scalar1=-1.0,
                                scalar2=None, op0=AL.mult)
        w16 = consts.tile([128, 2, O], FP16)
        nc.scalar.copy(w16, w32)
        bp16 = consts.tile([F, O], FP16)
        nc.scalar.copy(bp16, bp32)

        # long-lived per-sample tensors
        u16 = big.tile([P, SPC, NCH, C], FP16)      # normalized res
        or16 = big.tile([P, SPC, NCH, F], FP16)
        uT16 = big.tile([64, SPC, NN], FP16)
        oT16 = big.tile([F, SPC, NN], FP16)
        t4 = scal.tile([64, SPC, NT], FP32)         # per-group t partials
        t16 = scal.tile([64, SPC], FP16)
        acc_mx = [big.tile([P, NN], FP16, name=f"amx{s}") for s in range(SPC)]
        acc_mn = [big.tile([P, NN], FP16, name=f"amn{s}") for s in range(SPC)]
        gmax_r = scal.tile([P, SPC], FP32)
        neg_gmin = scal.tile([P, SPC], FP32)        # holds -gmin (>=0)
        inv_r = scal.tile([P, SPC], FP32)
        d2 = scal.tile([P, SPC, NCH], FP32)
        h2 = big.tile([P, SPC, NCH, F], FP16)
        xg1 = big.tile([P, SPC, NCH, F], FP16)
        mt0 = big.tile([128, SPC, NN], FP16)
        mt1 = big.tile([128, SPC, NN], FP16)

        def phase_B(s):
            sq = big.tile([P, NCH * C], FP32, name=f"sq{s}")
            rfl = res32[:, s].rearrange("p i c -> p (i c)")
            nc.vector.tensor_tensor(out=sq, in0=rfl, in1=rfl, op=AL.mult)
            nrm2 = scal.tile([P, NCH], FP32, name=f"nrm2{s}")
            nc.vector.tensor_reduce(nrm2, sq.rearrange("p (i c) -> p i c", i=NCH),
                                    axis=AX.X, op=AL.add)
            rn = scal.tile([P, NCH], FP32, name=f"rn{s}")
            nc.scalar.activation(out=rn, in_=nrm2, func=AF.Sqrt)
            nc.vector.reciprocal(out=rn, in_=rn)
            nc.vector.tensor_tensor(
                out=u16[:, s], in0=res32[:, s],
                in1=ap_view(rn, [[1, NCH], [0, C]]), op=AL.mult)
            nc.scalar.copy(or16[:, s], org32[:, s])

        def phase_C(s):
            # groups descending so the gram (chunks descending) can start early
            # transpose outputs use a 128-elem pitch (4B-aligned PSUM writes)
            for g in range(NT - 1, -1, -1):
                tp = ptp.tile([80, 512], FP16, tag="tp")
                for k in range(NT):
                    i = g * 4 + k
                    nc.tensor.transpose(tp[0:64, k * 128:k * 128 + P],
                                        u16[:, s, i, :], ident16)
                    nc.tensor.transpose(tp[64:80, k * 128:k * 128 + P],
                                        or16[:, s, i, :], ident16)
                sl = slice(g * BLK, (g + 1) * BLK)
                nc.scalar.activation(
                    out=ap_view(uT16[:, s, sl], [[P, 4], [1, P]]),
                    in_=ap_view(tp[0:64, :], [[128, 4], [1, P]]), func=AF.Copy,
                    accum_out=t4[:, s, g:g + 1])
                nc.scalar.activation(
                    out=ap_view(oT16[:, s, sl], [[P, 4], [1, P]]),
                    in_=ap_view(tp[64:80, :], [[128, 4], [1, P]]), func=AF.Copy)
            t2 = scal.tile([64, 1], FP32, name=f"t2{s}")
            nc.vector.tensor_reduce(t2, t4[:, s], axis=AX.X, op=AL.add)
            nc.vector.tensor_copy(t16[:, s:s + 1], t2)

        def phase_E(s):
            # gram + min/max scan, chunks descending
            for i in range(NCH - 1, -1, -1):
                js, d_off = i // 4, P * (i % 4)
                k = NT - js
                lhs = uT16[:, s, i * P:(i + 1) * P]
                g16 = sg.tile([P, NN], FP16, tag="g16")
                pos = 0
                for t in range((k + 1) // 2):
                    nb = min(2, k - 2 * t)
                    # 512-elem block pitch keeps each matmul out in one bank
                    gp = pg.tile([128, 2, 512], FP32, tag="gp")
                    for b in range(nb):
                        j = js + 2 * t + b
                        diag = (t == 0 and b == 0)
                        nc.tensor.matmul(gp[0:P, b, 0:BLK], lhs,
                                         uT16[:, s, j * BLK:(j + 1) * BLK],
                                         start=True, stop=not diag)
                        if diag:
                            nc.tensor.matmul(
                                gp[0:P, 0, d_off:d_off + P], ident16, negid16,
                                start=False, stop=True, skip_group_check=True)
                    for b in range(nb):
                        d0 = (d_off - (d_off & 1)) if (t == 0 and b == 0) else 0
                        w = BLK - d0
                        nc.scalar.activation(out=g16[:, pos:pos + w],
                                             in_=gp[0:P, b, d0:BLK], func=AF.Copy)
                        pos += w
                nc.vector.tensor_tensor(out=acc_mx[s][:, 0:pos],
                                        in0=acc_mx[s][:, 0:pos],
                                        in1=g16[:, 0:pos], op=AL.max)
                nc.vector.tensor_tensor(out=acc_mn[s][:, 0:pos],
                                        in0=acc_mn[s][:, 0:pos],
                                        in1=g16[:, 0:pos], op=AL.min)

        def phase_tail(s):
            # fold accs 2000->250, reduce, clamp vs 0, cross-partition reduce
            mx_p = scal.tile([P, 1], FP32, name=f"mxp{s}")
            mn_p = scal.tile([P, 1], FP32, name=f"mnp{s}")
            for acc, outp, opx in ((acc_mx[s], mx_p, AL.max),
                                   (acc_mn[s], mn_p, AL.min)):
                for w in (1000, 500, 250):
                    nc.vector.tensor_tensor(out=acc[:, 0:w], in0=acc[:, 0:w],
                                            in1=acc[:, w:2 * w], op=opx)
                nc.vector.tensor_reduce(outp, acc[:, 0:250], axis=AX.X, op=opx)
            nc.vector.tensor_scalar(out=mx_p, in0=mx_p, scalar1=0.0, scalar2=None,
                                    op0=AL.max)
            nc.vector.tensor_scalar(out=mn_p, in0=mn_p, scalar1=0.0, scalar2=-1.0,
                                    op0=AL.min, op1=AL.mult)
            nc.gpsimd.partition_all_reduce(gmax_r[:, s:s + 1], mx_p, channels=P,
                                           reduce_op=bass_isa.ReduceOp.max)
            nc.gpsimd.partition_all_reduce(neg_gmin[:, s:s + 1], mn_p, channels=P,
                                           reduce_op=bass_isa.ReduceOp.max)
            nc.vector.tensor_tensor(out=inv_r[:, s:s + 1], in0=gmax_r[:, s:s + 1],
                                    in1=neg_gmin[:, s:s + 1], op=AL.add)
            nc.vector.reciprocal(out=inv_r[:, s:s + 1], in_=inv_r[:, s:s + 1])

            # F: rowsum -> d
            rs_ps = ptl.tile([P, NCH], FP32, tag="tail", name=f"rs{s}")
            for i in range(NCH):
                nc.tensor.matmul(rs_ps[:, i:i + 1], uT16[:, s, i * P:(i + 1) * P],
                                 t16[:, s:s + 1], start=True, stop=True)
            bv = scal.tile([P, 1], FP32, name=f"bv{s}")
            nc.vector.tensor_scalar(out=bv, in0=neg_gmin[:, s:s + 1],
                                    scalar1=float(NN), scalar2=-1.0,
                                    op0=AL.mult, op1=AL.add)
            nc.vector.tensor_tensor(out=bv, in0=bv, in1=inv_r[:, s:s + 1],
                                    op=AL.mult)
            nc.scalar.activation(out=d2[:, s], in_=rs_ps, func=AF.Sqrt,
                                 scale=inv_r[:, s:s + 1], bias=bv)
            nc.vector.reciprocal(out=d2[:, s], in_=d2[:, s])

            # G: h, q, sv
            nc.vector.tensor_tensor(
                out=h2[:, s], in0=or16[:, s],
                in1=ap_view(d2[:, s], [[1, NCH], [0, F]]), op=AL.mult)
            q_ps = ptl.tile([64, F], FP32, tag="tail", name=f"q{s}")
            for i in range(NCH):
                nc.tensor.matmul(q_ps, u16[:, s, i, :], h2[:, s, i, :],
                                 start=(i == 0), stop=(i == NCH - 1))
            q16 = scal.tile([64, F], FP16, name=f"q16{s}")
            nc.vector.tensor_copy(q16, q_ps)
            sv_t = scal.tile([P, F], FP32, name=f"svt{s}")
            nc.vector.tensor_reduce(
                sv_t, ap_view(h2[:, s], [[1, F], [F, NCH]]), axis=AX.X, op=AL.add)
            sv_r = scal.tile([P, F], FP32, name=f"svr{s}")
            nc.gpsimd.partition_all_reduce(sv_r, sv_t, channels=P,
                                           reduce_op=bass_isa.ReduceOp.add)

            # H: v, xg1
            v_ps = ptl.tile([P, NCH, F], FP32, tag="tail", name=f"v{s}")
            for i in range(NCH):
                nc.tensor.matmul(v_ps[:, i, :], uT16[:, s, i * P:(i + 1) * P],
                                 q16, start=True, stop=True)
            gsv = scal.tile([P, F], FP32, name=f"gsv{s}")
            nc.vector.tensor_scalar(out=gsv, in0=sv_r,
                                    scalar1=neg_gmin[:, s:s + 1], scalar2=None,
                                    op0=AL.mult)
            xgf = big.tile([P, NCH, F], FP32, name=f"xgf{s}")
            nc.vector.tensor_tensor(out=xgf, in0=v_ps, in1=h2[:, s],
                                    op=AL.subtract)
            nc.vector.tensor_tensor(out=xgf, in0=xgf,
                                    in1=ap_view(gsv, [[0, NCH], [1, F]]),
                                    op=AL.add)
            dsc = scal.tile([P, NCH], FP32, name=f"dsc{s}")
            nc.vector.tensor_scalar(out=dsc, in0=d2[:, s],
                                    scalar1=inv_r[:, s:s + 1], scalar2=None,
                                    op0=AL.mult)
            nc.vector.tensor_tensor(out=xg1[:, s], in0=xgf,
                                    in1=ap_view(dsc, [[1, NCH], [0, F]]),
                                    op=AL.mult)

            # I: M build + transpose (s0 builds on gpsimd, s1 on DVE)
            eng = nc.gpsimd if s == 0 else nc.vector
            for g in range(NT):
                mg = sg.tile([P, 4, F, F], FP16, tag="mg")
                eng.tensor_tensor(
                    out=mg,
                    in0=ap_view(or16[:, s, g * 4:(g + 1) * 4, :],
                                [[F, 4], [1, F], [0, F]]),
                    in1=ap_view(xg1[:, s, g * 4:(g + 1) * 4, :],
                                [[F, 4], [0, F], [1, F]]),
                    op=AL.mult)
                mtp0 = ptl.tile([128, 512], FP16, tag="tail", name=f"mtp0_{s}_{g}")
                mtp1 = ptl.tile([128, 512], FP16, tag="tail", name=f"mtp1_{s}_{g}")
                for k in range(NT):
                    mf = mg[:, k].rearrange("p d i -> p (d i)")
                    nc.tensor.transpose(mtp0[:, k * 128:k * 128 + P],
                                        mf[:, 0:128], ident16)
                    nc.tensor.transpose(mtp1[:, k * 128:k * 128 + P],
                                        mf[:, 128:256], ident16)
                sl = slice(g * BLK, (g + 1) * BLK)
                nc.scalar.activation(
                    out=ap_view(mt0[:, s, sl], [[P, 4], [1, P]]),
                    in_=ap_view(mtp0[:, :], [[128, 4], [1, P]]), func=AF.Copy)
                nc.scalar.activation(
                    out=ap_view(mt1[:, s, sl], [[P, 4], [1, P]]),
                    in_=ap_view(mtp1[:, :], [[128, 4], [1, P]]), func=AF.Copy)

            # J: final matmuls with permuted moving operand (m = i*125+p,
            # streamed in node order n = p*16+i), tanh, store
            for t in range(NOT):
                ob = ptl.tile([O, OTW], FP32, tag="tail", name=f"ob{s}_{t}")
                pdims = [[1, 25], [125, F]]
                nc.tensor.matmul(ob, w16[:, 0, :],
                                 ap_view(mt0[:, s, 25 * t:], pdims),
                                 start=True, stop=False)
                nc.tensor.matmul(ob, w16[:, 1, :],
                                 ap_view(mt1[:, s, 25 * t:], pdims),
                                 start=False, stop=False)
                nc.tensor.matmul(ob, bp16,
                                 ap_view(oT16[:, s, 25 * t:], pdims),
                                 start=False, stop=True)
                ot32 = sg.tile([O, OTW], FP32, tag="ot32")
                nc.scalar.activation(out=ot32, in_=ob, func=AF.Tanh)
                nc.sync.dma_start(out=out_d[s, :, OTW * t:OTW * (t + 1)], in_=ot32)

        # acc init first: overlaps the input DMA dead time
        for s in range(SPC):
            nc.vector.memset(acc_mx[s], -2.0)
            nc.vector.memset(acc_mn[s], 2.0)
        phase_B(0)
        phase_C(0)
        phase_B(1)
        phase_C(1)
        phase_E(0)
        phase_E(1)
        phase_tail(0)
        phase_tail(1)
    return nc


_PROGRAM = None


def _get_program():
    global _PROGRAM
    if _PROGRAM is None:
        nc = bacc.Bacc("TRN2", target_bir_lowering=False, debug=False,
                       num_devices=NCORES)
        build_program(nc)
        nc.compile()
        _PROGRAM = nc
    return _PROGRAM


def kernel(**inputs):
    from concourse.bass_utils import run_bass_kernel_spmd
    res = np.asarray(inputs["res_x"], dtype=np.float32)
    org = np.asarray(inputs["origin_x"], dtype=np.float32)
    wp = np.asarray(inputs["weights_pool_x"], dtype=np.float32)
    bpl = np.asarray(inputs["bias_pool_x"], dtype=np.float32)
    nc = _get_program()
    in_maps = [
        {"res": res[c * SPC:(c + 1) * SPC], "origin": org[c * SPC:(c + 1) * SPC],
         "wpool": wp, "bpool": bpl}
        for c in range(NCORES)
    ]
    r = run_bass_kernel_spmd(nc, in_maps, list(range(NCORES)))
    out = np.concatenate([r.results[c]["out"] for c in range(NCORES)], axis=0)
    return out.astype(np.float32)
